# revision 1
# baseline (speedup 1.0000x reference)
"""Multi-head attention (B=2, S=2048, E=1024, H=16) on 8 trn2 NeuronCores.

Sharding: tensor-parallel over heads (2 heads per core).  Each core computes
q/k/v for its 2 heads from the full x, runs attention, and produces a partial
output projection (row-split w_proj); the host sums the 8 partials and adds
b_proj plus the constant b_v @ w_proj row.

All matmuls are bf16: fp8(e4m3) was measured to add ~2.2% relative noise
per quantization step, which softmax averaging passes ~1:1 into the output
(sim: fp8 attn@V alone = 2.9% > the 2e-2 gate), so fp8/DoubleRow is
numerically unusable here.  At bf16 the kernel runs at the PE moving-data
port floor (~84 MB per core / ~360 GB/s ~= 228 us):

  - scores per head are K=64 row-disjoint pairs (tile_position h*64) that
    co-execute on the PE, restoring full port width.
  - v is computed directly in [t, d] layout (no PE transpose phase).
  - global software pipeline over a flat (b, qt, tt) stream: attn@V
    trails scores/exp by 5 slots across q-tile/batch boundaries; psO
    tiles are allocated lazily at the first attn@V so pool-reuse waits
    never block the scores.
  - normalization per q-tile writes its own aoT tile (no false whole-
    tile dependency) using reciprocal_approx_fast (input bounced through
    sbuf: custom DVE ops cannot read PSUM) + Pool partition_broadcast.
  - the previous q-tile's projection is spread one chunk per slot so its
    psum evacuations (DVE) never bunch up; batch 1's qkv/v groups are
    interleaved into batch 0's attention stream.

Scaling: q~ = 16*(q+b) bf16 (w_qkv*16 on host), scores psum = 256*score,
exp scale (1/sqrt(64))/256 with bias -2 (softmax shift invariance; keeps
exp in comfortable range), v~ = 16*v, wp~ = w_proj/16.  v-bias and b_proj
are folded host-side into the gather: y += b_v @ w_proj + b_proj.

Measured: 223415 ns (baseline 314733 ns), rel err 0.0061.
"""

import ml_dtypes
import numpy as np

import concourse.bass as bass
import concourse.mybir as mybir
import concourse.tile as tile
from concourse import bacc
from concourse.bass_utils import run_bass_kernel_spmd

F32 = mybir.dt.float32
BF16 = mybir.dt.bfloat16
NPBF16 = ml_dtypes.bfloat16

E = 1024
NH = 16
DH = 64
NCORES = 8
HPC = NH // NCORES  # heads per core = 2
LF = HPC * DH  # local features per core = 128
NCHUNK = E // 128  # contraction chunks for the qkv projection = 8


def build_nc(B=2, S=2048):
    ST = 512  # q-tile width
    SH = S // 2  # s-half processed per xT load
    NST = SH // ST  # s-tiles per half = 2
    NTT = S // 128  # 128-row t-chunks per batch = 16
    NQ = S // ST  # q-tiles per batch = 4
    BS = B * S

    nc = bacc.Bacc("TRN2")
    xT = nc.dram_tensor("xT", [E, BS], BF16, kind="ExternalInput")
    wq = nc.dram_tensor("wq", [E, LF], BF16, kind="ExternalInput")
    wk = nc.dram_tensor("wk", [E, LF], BF16, kind="ExternalInput")
    wv = nc.dram_tensor("wv", [E, LF], BF16, kind="ExternalInput")
    bq = nc.dram_tensor("bq", [LF, 1], F32, kind="ExternalInput")
    bk = nc.dram_tensor("bk", [LF, 1], F32, kind="ExternalInput")
    wp = nc.dram_tensor("wp", [LF, E], BF16, kind="ExternalInput")
    y = nc.dram_tensor("y", [BS, E], F32, kind="ExternalOutput")

    mm = nc.tensor.matmul
    EXP_SCALE = (DH ** -0.5) / 256.0

    with tile.TileContext(nc) as tc:
        with (
            tc.tile_pool(name="consts", bufs=1) as consts,
            tc.tile_pool(name="xpool", bufs=3) as xpool,
            tc.tile_pool(name="acts", bufs=2) as acts,  # qT/kT bf16
            tc.tile_pool(name="vap", bufs=2) as vap,  # v2 [t,d] bf16
            tc.tile_pool(name="attp", bufs=7) as attp,  # a exp bf16
            tc.tile_pool(name="aop", bufs=3) as aop,  # per-qt aoT tiles
            tc.tile_pool(name="npool", bufs=3) as npool,
            tc.tile_pool(name="ypool", bufs=4) as ypool,
            tc.tile_pool(name="psA", bufs=2, space="PSUM") as psA,
            tc.tile_pool(name="psS", bufs=2, space="PSUM") as psS,
            tc.tile_pool(name="psO", bufs=2, space="PSUM") as psO,
        ):
            # ---- constants ----
            wq_sb = consts.tile([128, NCHUNK, LF], BF16, tag="wq")
            wk_sb = consts.tile([128, NCHUNK, LF], BF16, tag="wk")
            wv_sb = consts.tile([128, NCHUNK, LF], BF16, tag="wv")
            nc.sync.dma_start(out=wq_sb, in_=wq.rearrange("(c p) n -> p c n", p=128))
            nc.sync.dma_start(out=wk_sb, in_=wk.rearrange("(c p) n -> p c n", p=128))
            nc.sync.dma_start(out=wv_sb, in_=wv.rearrange("(c p) n -> p c n", p=128))
            wp_sb = consts.tile([LF, E], BF16, tag="wp")
            nc.sync.dma_start(out=wp_sb, in_=wp[:, :])
            bq_sb = consts.tile([LF, 1], F32, tag="bq")
            bk_sb = consts.tile([LF, 1], F32, tag="bk")
            nc.sync.dma_start(out=bq_sb, in_=bq[:, :])
            nc.sync.dma_start(out=bk_sb, in_=bk[:, :])
            expb_sb = consts.tile([128, 1], F32, tag="expb")
            nc.vector.memset(expb_sb, -2.0)

            xT_r = xT.rearrange("(c p) s -> p c s", p=128)

            # per-batch state
            qTs, kTs, v2s, aoTs, xts = {}, {}, {}, {}, {}

            def ensure_x(b, sh):
                if (b, sh) not in xts:
                    xt_new = xpool.tile(
                        [128, NCHUNK, SH], BF16, tag="xt", name=f"xt{b}{sh}"
                    )
                    s0 = b * S + sh * SH
                    nc.sync.dma_start(out=xt_new, in_=xT_r[:, :, s0 : s0 + SH])
                    xts[(b, sh)] = xt_new
                return xts[(b, sh)]

            def emit_qk_group(b, sh, which):
                """One (s-half, q|k) block of the projection -> bf16."""
                if b not in qTs:
                    qTs[b] = acts.tile([128, S], BF16, tag="qT", name=f"qT{b}")
                    kTs[b] = acts.tile([128, S], BF16, tag="kT", name=f"kT{b}")
                dst, w_sb, b_sb = {
                    "q": (qTs[b], wq_sb, bq_sb),
                    "k": (kTs[b], wk_sb, bk_sb),
                }[which]
                xt_sb = ensure_x(b, sh)
                for st in range(NST):
                    ps = psA.tile([128, ST], F32, tag="psA")
                    for c in range(NCHUNK):
                        mm(
                            ps,
                            lhsT=w_sb[:, c, :],
                            rhs=xt_sb[:, c, st * ST : (st + 1) * ST],
                            start=(c == 0),
                            stop=(c == NCHUNK - 1),
                        )
                    g0 = sh * SH + st * ST
                    # evac: psum (=16*x@w) + b~ -> bf16 q~ = 16*(q+b)
                    nc.vector.tensor_scalar_add(dst[:, g0 : g0 + ST], ps, b_sb)

            def emit_v_group(b, sh):
                """v~ = 16*v in [t, d] bf16 layout directly (v-direct)."""
                if b not in v2s:
                    v2s[b] = vap.tile(
                        [128, NTT, HPC, DH + 1], BF16, tag="v2", name=f"v2{b}"
                    )
                    # col DH = ones (denominator row)
                    nc.gpsimd.memset(v2s[b][:, :, :, DH : DH + 1], 1.0)
                v2 = v2s[b]
                xt_sb = ensure_x(b, sh)
                for sc in range(SH // 128):
                    scg = sh * (SH // 128) + sc  # global t-chunk id
                    psv = psA.tile([128, HPC, DH], F32, tag="psA")
                    for c in range(NCHUNK):
                        mm(
                            psv,
                            lhsT=xt_sb[:, c, sc * 128 : (sc + 1) * 128],
                            rhs=wv_sb[:, c, :],
                            start=(c == 0),
                            stop=(c == NCHUNK - 1),
                        )
                    # evac psum (=16*v) -> v~ = 16*v in bf16
                    nc.vector.tensor_copy(v2[:, scg, :, 0:DH], psv)


            def emit_sc(b, qt, tt, a_tiles):
                """Scores + exp for one (q-tile, t-chunk), per head."""
                qT, kT = qTs[b], kTs[b]
                qsl = slice(qt * ST, (qt + 1) * ST)
                tsl = slice(tt * 128, (tt + 1) * 128)
                ps_s = psS.tile([128, HPC * ST], F32, tag="psS")
                for h in range(HPC):
                    hsl = slice(h * DH, (h + 1) * DH)
                    mm(
                        ps_s[:, h * ST : (h + 1) * ST],
                        lhsT=kT[hsl, tsl],
                        rhs=qT[hsl, qsl],
                        start=True,
                        stop=True,
                        tile_position=(h * DH, 0),
                    )
                a = attp.tile([128, HPC, ST], BF16, tag="a")
                a_tiles[tt] = a
                nc.scalar.activation(
                    a,
                    ps_s,
                    mybir.ActivationFunctionType.Exp,
                    bias=expb_sb,
                    scale=EXP_SCALE,
                )

            def emit_av(b, qt, tt, a_tiles, out_ps):
                """attn@V for one t-chunk; lazily allocates the psO tiles so
                their pool-reuse wait lands here, not on earlier scores."""
                v2 = v2s[b]
                if tt == 0:
                    for h in range(HPC):
                        out_ps.append(
                            psO.tile([128, ST], F32, tag="psO", name=f"psO_{h}")
                        )
                a = a_tiles[tt] if tt in a_tiles else a_tiles.pop(tt)
                for h in range(HPC):
                    mm(
                        out_ps[h][0 : DH + 1, :],
                        lhsT=v2[:, tt, h, :],
                        rhs=a[:, h, :],
                        start=(tt == 0),
                        stop=(tt == NTT - 1),
                    )
                a_tiles.pop(tt, None)

            def emit_norm_qt(b, qt, out_ps):
                """Normalize this q-tile (denominator row DH of each psO).
                Writes a per-q-tile aoT tile so the projection of the
                previous q-tile has no false tile dependency on it."""
                aoT = aop.tile([128, ST], BF16, tag="aoT", name=f"ao{b}_{qt}")
                aoTs[(b, qt)] = aoT
                dens, recs, bcs = [], [], []
                for h in range(HPC):
                    den_sb = npool.tile([1, ST], F32, tag="den")
                    nc.vector.tensor_copy(den_sb, out_ps[h][DH : DH + 1, :])
                    dens.append(den_sb)
                for h in range(HPC):
                    rec = npool.tile([1, ST], F32, tag="rec")
                    nc.vector.reciprocal_approx_fast(rec, dens[h])
                    recs.append(rec)
                    bc_sb = npool.tile([DH, ST], F32, tag="bc")
                    nc.gpsimd.partition_broadcast(bc_sb, rec)
                    bcs.append(bc_sb)
                for h in range(HPC):
                    nc.vector.tensor_mul(
                        aoT[h * DH : (h + 1) * DH, :],
                        out_ps[h][0:DH, :],
                        bcs[h],
                    )

            def emit_proj_chunk(b, qt, st, eh):
                """One (s128, e512) chunk of a q-tile's output projection."""
                aoT = aoTs[(b, qt)]
                s_loc = qt * ST + st * 128
                r0 = b * S + s_loc
                esl = slice(eh * 512, (eh + 1) * 512)
                ps_y = psA.tile([128, 512], F32, tag="psA")
                mm(
                    ps_y,
                    lhsT=aoT[:, st * 128 : (st + 1) * 128],
                    rhs=wp_sb[:, esl],
                    start=True,
                    stop=True,
                )
                y_sb = ypool.tile([128, 512], F32, tag="y")
                nc.vector.tensor_copy(y_sb, ps_y)
                nc.sync.dma_start(out=y[r0 : r0 + 128, esl], in_=y_sb)

            # ---- emission schedule ----
            def emit_A(b):
                for sh in range(2):
                    emit_qk_group(b, sh, "q")
                    emit_qk_group(b, sh, "k")
                    emit_v_group(b, sh)

            emit_A(0)
            # interleave batch 1's A-phase into batch 0's attention
            items = [
                ("qk", 0, "q"), ("v", 0, None), ("qk", 0, "k"),
                ("qk", 1, "q"), ("v", 1, None), ("qk", 1, "k"),
            ]
            per_qt = -(-len(items) // NQ)
            interleave = {
                qt: items[qt * per_qt : (qt + 1) * per_qt] for qt in range(NQ)
            }
            # Global software pipeline: flat (b, qt, tt) stream; attn@V
            # trails the scores/exp by DELAY slots across q-tile and batch
            # boundaries; the previous q-tile's projection chunks are spread
            # one per slot so their psum evacuations never bunch up.
            DELAY = 5
            seq = [
                (b, qt, tt)
                for b in range(B)
                for qt in range(NQ)
                for tt in range(NTT)
            ]
            a_tiles = {}
            qt_state = {}  # (b, qt) -> out_ps list
            from collections import deque
            pending_proj = deque()

            def boundary(bq, qq, g):
                """av of (bq, qq) just completed: norm now, queue proj
                (first chunk held back 3 slots so it lands after the
                normalization chain has finished)."""
                emit_norm_qt(bq, qq, qt_state.pop((bq, qq)))
                for st in range(ST // 128):
                    for eh in range(E // 512):
                        pending_proj.append((g + 3, (bq, qq, st, eh)))
                if bq + 1 < B:
                    for item in interleave.get(qq, []):
                        kind, sh, which = item
                        if kind == "qk":
                            emit_qk_group(bq + 1, sh, which)
                        else:
                            emit_v_group(bq + 1, sh)

            for g, (b, qt, tt) in enumerate(seq):
                emit_sc(b, qt, tt, a_tiles)
                if g >= DELAY:
                    bb, qb, tb = seq[g - DELAY]
                    ops = qt_state.setdefault((bb, qb), [])
                    emit_av(bb, qb, tb, a_tiles, ops)
                    if tb == NTT - 1:
                        boundary(bb, qb, g)
                if pending_proj and pending_proj[0][0] <= g:
                    emit_proj_chunk(*pending_proj.popleft()[1])
            for g in range(len(seq) - DELAY, len(seq)):
                bb, qb, tb = seq[g]
                ops = qt_state.setdefault((bb, qb), [])
                emit_av(bb, qb, tb, a_tiles, ops)
                if tb == NTT - 1:
                    boundary(bb, qb, g)
            while pending_proj:
                emit_proj_chunk(*pending_proj.popleft()[1])

    nc.compile()
    return nc


_NC_CACHE = {}


def _get_nc(B, S):
    key = (B, S)
    if key not in _NC_CACHE:
        _NC_CACHE[key] = build_nc(B, S)
    return _NC_CACHE[key]


def make_in_maps(x, w_qkv, b_qkv, w_proj):
    B, S, _ = x.shape
    xT = np.ascontiguousarray(x.reshape(B * S, E).T).astype(NPBF16)
    in_maps = []
    for c in range(NCORES):
        cols = slice(c * LF, (c + 1) * LF)
        in_maps.append(
            {
                "xT": xT,
                "wq": np.ascontiguousarray(
                    w_qkv[:, 0 * E : 1 * E][:, cols] * 16.0
                ).astype(NPBF16),
                "wk": np.ascontiguousarray(
                    w_qkv[:, 1 * E : 2 * E][:, cols] * 16.0
                ).astype(NPBF16),
                "wv": np.ascontiguousarray(
                    w_qkv[:, 2 * E : 3 * E][:, cols] * 16.0
                ).astype(NPBF16),
                "bq": (b_qkv[0 * E : 1 * E][cols] * 16.0)
                .reshape(LF, 1)
                .astype(np.float32),
                "bk": (b_qkv[1 * E : 2 * E][cols] * 16.0)
                .reshape(LF, 1)
                .astype(np.float32),
                "wp": np.ascontiguousarray(w_proj[cols, :] / 16.0).astype(
                    NPBF16
                ),
            }
        )
    return in_maps


def kernel_run(x, w_qkv, b_qkv, w_proj, b_proj, trace=False):
    x = np.asarray(x, dtype=np.float32)
    w_qkv = np.asarray(w_qkv, dtype=np.float32)
    b_qkv = np.asarray(b_qkv, dtype=np.float32)
    w_proj = np.asarray(w_proj, dtype=np.float32)
    b_proj = np.asarray(b_proj, dtype=np.float32)
    B, S, _ = x.shape
    nc = _get_nc(B, S)
    in_maps = make_in_maps(x, w_qkv, b_qkv, w_proj)
    res = run_bass_kernel_spmd(
        nc, in_maps, core_ids=list(range(NCORES)), trace=trace
    )
    y = res.results[0]["y"].astype(np.float64)
    for c in range(1, NCORES):
        y += res.results[c]["y"]
    # v-bias contribution: (b_v @ w_proj) constant row + b_proj
    bv = b_qkv[2 * E : 3 * E]
    y += (bv @ w_proj + b_proj)[None, :]
    return y.astype(np.float32).reshape(B, S, E), res


def kernel(x, w_qkv, b_qkv, w_proj, b_proj):
    y, _ = kernel_run(x, w_qkv, b_qkv, w_proj, b_proj)
    return y

